# revision 18
# baseline (speedup 1.0000x reference)
"""Trainium2 Bass kernel for AxialMHA (B=2, N=2048, D=1024, H=16, dh=64).

Sharding: tensor-parallel over heads - 16 heads / 8 cores = 2 heads per core.
Each core computes q/k/v projections for its 2 heads (full batch), runs
attention, and produces a partial output projection (contraction over its
128 feature dims). Host sums the 8 partials and adds the effective bias
(bv @ Wproj + bproj - the v-bias commutes through softmax-weighted sums).

v3: the kernel is one global stream of 128 scores+exp "units" (ACT's exp
stream is the secondary bottleneck at ~134us busy vs PE ~137us); all other
work - stage-A projection chunks, AV accumulation groups, normalize+
transpose, output-projection columns - is chopped into sub-microsecond
"pieces" drained between units once their upstream exps are guaranteed
complete (readiness tracked by unit index, max 2 pieces per slot). This
keeps both in-order engine queues free of head-of-line blocking:
- AV consumes exp'd score tiles as the stationary operand streaming
  [V_h | ones] (65 cols) per 128-qtok group; denominators ride in col 64.
- Normalize on DVE; transpose back to feature-major via PE identity-matmul
  into psum banks time-shared with the AV accumulators.
"""

import os
import sys

import numpy as np
import ml_dtypes

for _p in ("/opt/trn_rl_repo",):
    if _p not in sys.path and os.path.isdir(_p):
        sys.path.insert(0, _p)

import concourse.bass as bass
import concourse.tile as tile
from concourse import bacc, masks, mybir
from concourse.bass_utils import run_bass_kernel_spmd

BF16 = mybir.dt.bfloat16
F32 = mybir.dt.float32
AF = mybir.ActivationFunctionType

B, N, D, H, DH = 2, 2048, 1024, 16, 64
NC = 8            # cores
HC = H // NC      # heads per core = 2
TOK = B * N       # 4096
CH = 8            # token chunks of 512 for projections
CW = TOK // CH    # 512
KTD = D // 128    # 8 contraction tiles for projections
NKT = N // 128    # 16 ktok tiles per batch
QC = N // 512     # 4 qchunks per batch
DV = DH + 1       # V cols + ones column for softmax denominators
EXP_SLACK = int(os.environ.get("KV_SLACK", 5))     # units of lag assumed between score emission and exp done


def build_nc():
    nc = bacc.Bacc(
        "TRN2",
        target_bir_lowering=False,
        debug=False,
        enable_asserts=False,
        num_devices=NC,
    )
    xT = nc.dram_tensor("xT", [D, TOK], BF16, kind="ExternalInput").ap()
    wq = nc.dram_tensor("wq", [D, 128], BF16, kind="ExternalInput").ap()
    wk = nc.dram_tensor("wk", [D, 128], BF16, kind="ExternalInput").ap()
    wv = nc.dram_tensor("wv", [D, 128], BF16, kind="ExternalInput").ap()
    wo = nc.dram_tensor("wo", [128, D], BF16, kind="ExternalInput").ap()
    bq = nc.dram_tensor("bq", [128, 1], F32, kind="ExternalInput").ap()
    bk = nc.dram_tensor("bk", [128, 1], F32, kind="ExternalInput").ap()
    out_p = nc.dram_tensor("out_p", [D, TOK], BF16, kind="ExternalOutput").ap()

    from contextlib import ExitStack

    with tile.TileContext(nc) as tc, ExitStack() as ctx:
        singles = ctx.enter_context(tc.tile_pool(name="singles", bufs=1))

        bq_sb = singles.tile([128, 1], F32)
        bk_sb = singles.tile([128, 1], F32)
        wq_sb = singles.tile([128, KTD, 128], BF16)
        wk_sb = singles.tile([128, KTD, 128], BF16)
        wv_sb = singles.tile([128, KTD, 128], BF16)
        wo_sb = singles.tile([128, D], BF16)
        identity = singles.tile([128, 128], BF16)
        masks.make_identity(nc, identity[:])

        QT = [[singles.tile([128, CW], BF16, name=f"QT{b}_{q}") for q in range(QC)]
              for b in range(B)]
        KT = [[singles.tile([128, CW], BF16, name=f"KT{b}_{q}") for q in range(QC)]
              for b in range(B)]
        V1 = [[singles.tile([128, 4, HC, DV], BF16, name=f"V1{b}_{q}")
               for q in range(QC)] for b in range(B)]
        for b in range(B):
            for q in range(QC):
                nc.vector.memset(V1[b][q], 1.0)
        yT = [singles.tile([128, N], BF16, name=f"yT{b}") for b in range(B)]

        ets = {}  # (b, qc, h, ktg) -> et tile
        pools = {}

        # ---- emission helpers -------------------------------------------
        def chunk_qk(b, cc, first=False):
            xpool, psA = pools["xp"], pools["psA"]
            c = b * (CH // B) + cc
            xt = xpool.tile([128, KTD, CW], BF16, tag="xt", name="xt")
            xs = xT[:, c * CW:(c + 1) * CW].rearrange("(ko p) n -> p ko n", p=128)
            if first:
                for qq in range(4):
                    nc.sync.dma_start(xt[:, 2 * qq:2 * qq + 2, :],
                                      xs[:, 2 * qq:2 * qq + 2, :])
                nc.sync.dma_start(wk_sb, wk.rearrange("(ko p) m -> p ko m", p=128))
                nc.sync.dma_start(wv_sb, wv.rearrange("(ko p) m -> p ko m", p=128))
            else:
                nc.sync.dma_start(xt[:, 0:KTD // 2, :], xs[:, 0:KTD // 2, :])
                nc.sync.dma_start(xt[:, KTD // 2:, :], xs[:, KTD // 2:, :])
            pq = psA.tile([128, CW], F32, tag="pqk", name="pq")
            for k in range(KTD):
                nc.tensor.matmul(pq, lhsT=wq_sb[:, k, :], rhs=xt[:, k, :],
                                 start=(k == 0), stop=(k == KTD - 1))
            nc.vector.tensor_tensor(QT[b][cc], pq,
                                    bq_sb.to_broadcast((128, CW)),
                                    mybir.AluOpType.add)
            pk = psA.tile([128, CW], F32, tag="pqk", name="pk")
            for k in range(KTD):
                nc.tensor.matmul(pk, lhsT=wk_sb[:, k, :], rhs=xt[:, k, :],
                                 start=(k == 0), stop=(k == KTD - 1))
            nc.vector.tensor_tensor(KT[b][cc], pk,
                                    bk_sb.to_broadcast((128, CW)),
                                    mybir.AluOpType.add)
            return xt

        def chunk_v(b, cc, xt):
            psV = pools["psV"]
            pv = psV.tile([128, 4, 128], F32, tag="pp", name="pv")
            for s in range(CW // 128):
                for k in range(KTD):
                    nc.tensor.matmul(pv[:, s, :],
                                     lhsT=xt[:, k, s * 128:(s + 1) * 128],
                                     rhs=wv_sb[:, k, :],
                                     start=(k == 0), stop=(k == KTD - 1))
            for h in range(HC):
                nc.vector.tensor_copy(V1[b][cc][:, :, h, 0:DH],
                                      pv[:, :, h * DH:(h + 1) * DH])

        def sc_unit(b, qc, h, ktg):
            stp, epool = pools["stp"], pools["ep"]
            hs = slice(h * DH, (h + 1) * DH)
            stt = stp.tile([128, 2, 512], F32, tag="st", name="stt")
            for j in range(2):
                kt = ktg * 2 + j
                kc, ks = divmod(kt, 4)
                nc.tensor.matmul(
                    stt[:, j, :],
                    lhsT=KT[b][kc][hs, ks * 128:(ks + 1) * 128],
                    rhs=QT[b][qc][hs, :],
                    start=True, stop=True,
                    tile_position=(h * DH, 0),
                )
            et = epool.tile([128, 2, 512], BF16, tag="et", name="et")
            nc.scalar.activation(et, stt, AF.Exp, scale=0.125)
            ets[(b, qc, h, ktg)] = et

        def av_group(b, qc, h, qt, py):
            for kt in range(NKT):
                kc, ks = divmod(kt, 4)
                nc.tensor.matmul(
                    py[:, qt, 0:DV],
                    lhsT=ets[(b, qc, h, kt // 2)][:, kt % 2,
                                                  qt * 128:(qt + 1) * 128],
                    rhs=V1[b][kc][:, ks, h, :],
                    start=(kt == 0), stop=(kt == NKT - 1))

        def av_norm(b, qc, h, py):
            rpool, npool, pyp = pools["rp"], pools["np"], pools["pyp"]
            qo = qc * 512
            rsb = rpool.tile([128, 4, 1], F32, tag="rsb", name="rsb")
            nc.vector.reciprocal(rsb, py[:, :, DH:DV])
            yn = npool.tile([128, 4, DH], BF16, tag="yn", name="yn")
            nc.vector.tensor_tensor(yn, py[:, :, 0:DH],
                                    rsb.to_broadcast((128, 4, DH)),
                                    mybir.AluOpType.mult)
            ytr = pyp.tile([128, 4, 128], F32, tag=f"py{h}", name=f"ytr{h}")
            for qt in range(4):
                # transpose as a plain matmul: yn.T @ I  ->  [64 x 128]
                nc.tensor.matmul(ytr[0:DH, qt, :], lhsT=yn[:, qt, :],
                                 rhs=identity, start=True, stop=True)
            nc.vector.tensor_copy(yT[b][h * DH:(h + 1) * DH, qo:qo + 512],
                                  ytr[0:DH, 0:4, :])

        def av_norm_half(b, qc, h, py, half, state):
            # last chunk: normalize/transpose/copy one 256-token half so the
            # output projection and stores start before the other half's AV
            rpool, npool = pools["rp"], pools["np"]
            qo = qc * 512
            q0 = 2 * half
            rsb = rpool.tile([128, 2, 1], F32, tag="rs2", name="rs2")
            nc.vector.reciprocal(rsb, py[:, q0:q0 + 2, DH:DV])
            yn = npool.tile([128, 2, DH], BF16, tag="yn2", name="yn2")
            nc.vector.tensor_tensor(yn, py[:, q0:q0 + 2, 0:DH],
                                    rsb.to_broadcast((128, 2, DH)),
                                    mybir.AluOpType.mult)
            if half == 0:
                state["ytr"] = pools["stp"].tile([128, 8, 128], F32,
                                                 tag="st", name="ytr_l")
            ytr = state["ytr"]
            for qt in (q0, q0 + 1):
                nc.tensor.matmul(ytr[0:DH, qt, :], lhsT=yn[:, qt - q0, :],
                                 rhs=identity, start=True, stop=True)
            nc.vector.tensor_copy(
                yT[b][h * DH:(h + 1) * DH, qo + half * 256:qo + half * 256 + 256],
                ytr[0:DH, q0:q0 + 2, :])

        def av_piece_list(b, qc, h, split=False):
            state = {}

            def alloc_g0():
                state["py"] = pools["pyp"].tile(
                    [128, 4, 128], F32, tag=f"py{h}", name=f"py{h}")
                av_group(b, qc, h, 0, state["py"])

            if split:
                return [
                    alloc_g0,
                    lambda: av_group(b, qc, h, 1, state["py"]),
                    lambda: av_norm_half(b, qc, h, state["py"], 0, state),
                    lambda: av_group(b, qc, h, 2, state["py"]),
                    lambda: av_group(b, qc, h, 3, state["py"]),
                    lambda: av_norm_half(b, qc, h, state["py"], 1, state),
                ]
            return [
                alloc_g0,
                lambda: av_group(b, qc, h, 1, state["py"]),
                lambda: av_group(b, qc, h, 2, state["py"]),
                lambda: (av_group(b, qc, h, 3, state["py"]),
                         av_norm(b, qc, h, state["py"])),
            ]

        proj_ps = {}
        proj_pp = {}

        def proj_piece(b, cc, ot, pool_copy=False, wide=False):
            # ot pairs (0,1), (2,3), ... share one staging tile and one DMA
            cs = slice(cc * CW, (cc + 1) * CW)
            if ot % 2 == 0:
                pp = pools["psV"].tile([128, CW], F32, tag="pp", name="pp")
            else:
                pp = pools["psA"].tile([128, CW], F32, tag="pqk", name="pp")
            nc.tensor.matmul(pp, lhsT=wo_sb[:, ot * 128:(ot + 1) * 128],
                             rhs=yT[b][:, cs],
                             start=True, stop=True)
            if ot % 2 == 0:
                ps = pools["pp"].tile([128, 2, CW], BF16, tag="ps", name="ps")
                proj_ps[(b, cc)] = ps
            else:
                ps = proj_ps[(b, cc)]
            if pool_copy:
                nc.scalar.copy(ps[:, ot % 2, :], pp)
            else:
                nc.vector.tensor_copy(ps[:, ot % 2, :], pp)
            if ot % 2 == 1:
                po = ot - 1
                dst = out_p[po * 128:(po + 2) * 128,
                            b * N + cc * CW:b * N + (cc + 1) * CW]
                nc.sync.dma_start(dst.rearrange("(o p) n -> p o n", p=128), ps)

        def proj_piece_half(b, cc, ot, half):
            # last chunk: 256-token halves, stores fire per (pair, half)
            ts = slice(cc * CW + half * 256, cc * CW + half * 256 + 256)
            if ot % 2 == 0:
                pp = pools["psV"].tile([128, CW], F32, tag="pp", name="pp")
            else:
                pp = pools["psA"].tile([128, CW], F32, tag="pqk", name="pp")
            nc.tensor.matmul(pp[:, 0:256],
                             lhsT=wo_sb[:, ot * 128:(ot + 1) * 128],
                             rhs=yT[b][:, ts],
                             start=True, stop=True)
            if ot % 2 == 0 and half == 0:
                ps = pools["pp"].tile([128, 2, CW], BF16, tag="ps", name="ps")
                proj_ps[(b, cc, ot // 2)] = ps
            else:
                ps = proj_ps[(b, cc, ot // 2)]
            if ot % 2 == 1:
                nc.scalar.copy(ps[:, 1, half * 256:half * 256 + 256],
                               pp[:, 0:256])
            else:
                nc.vector.tensor_copy(ps[:, 0, half * 256:half * 256 + 256],
                                      pp[:, 0:256])
            if ot % 2 == 1:
                po = ot - 1
                dst = out_p[po * 128:(po + 2) * 128, b * N + ts.start:
                            b * N + ts.stop]
                nc.sync.dma_start(dst.rearrange("(o p) n -> p o n", p=128),
                                  ps[:, :, half * 256:half * 256 + 256])

        # ---- split stage-A chunk emitters for lane scheduling -----------
        xts = {}

        def a_pk(b, c, first=False):
            xpool, psA = pools["xp"], pools["psA"]
            cg = b * (CH // B) + c
            xt = xpool.tile([128, KTD, CW], BF16, tag="xt", name="xt")
            xts[(b, c)] = xt
            xs = xT[:, cg * CW:(cg + 1) * CW].rearrange(
                "(ko p) n -> p ko n", p=128)
            nc.sync.dma_start(xt[:, 0:KTD // 2, :], xs[:, 0:KTD // 2, :])
            nc.sync.dma_start(xt[:, KTD // 2:, :], xs[:, KTD // 2:, :])
            pk = psA.tile([128, CW], F32, tag="pqk", name="pk")
            for k in range(KTD):
                nc.tensor.matmul(pk, lhsT=wk_sb[:, k, :], rhs=xt[:, k, :],
                                 start=(k == 0), stop=(k == KTD - 1))
            nc.vector.tensor_tensor(KT[b][c], pk,
                                    bk_sb.to_broadcast((128, CW)),
                                    mybir.AluOpType.add)

        def a_pq(b, c):
            psA = pools["psA"]
            xt = xts[(b, c)]
            pq = psA.tile([128, CW], F32, tag="pqk", name="pq")
            for k in range(KTD):
                nc.tensor.matmul(pq, lhsT=wq_sb[:, k, :], rhs=xt[:, k, :],
                                 start=(k == 0), stop=(k == KTD - 1))
            nc.vector.tensor_tensor(QT[b][c], pq,
                                    bq_sb.to_broadcast((128, CW)),
                                    mybir.AluOpType.add)

        def a_pv(b, c, half):
            psV = pools["psV"]
            xt = xts[(b, c)]
            pv = psV.tile([128, 2, 128], F32, tag="pp", name="pv")
            for si, s in enumerate((2 * half, 2 * half + 1)):
                for k in range(KTD):
                    nc.tensor.matmul(pv[:, si, :],
                                     lhsT=xt[:, k, s * 128:(s + 1) * 128],
                                     rhs=wv_sb[:, k, :],
                                     start=(k == 0), stop=(k == KTD - 1))
            for h in range(HC):
                nc.vector.tensor_copy(
                    V1[b][c][:, 2 * half:2 * half + 2, h, 0:DH],
                    pv[:, :, h * DH:(h + 1) * DH])

        # ---- full emission: EDF-woven unit/piece stream -----------------
        with tc.tile_pool(name="xp", bufs=4) as xpool, \
             tc.tile_pool(name="psA", bufs=1, space="PSUM") as psA, \
             tc.tile_pool(name="psV", bufs=1, space="PSUM") as psV, \
             tc.tile_pool(name="stp", bufs=2, space="PSUM") as stp, \
             tc.tile_pool(name="pyp", bufs=1, space="PSUM") as pyp, \
             tc.tile_pool(name="ep", bufs=int(os.environ.get("KV_EP", 46))) as epool, \
             tc.tile_pool(name="rp", bufs=4) as rpool, \
             tc.tile_pool(name="np", bufs=4) as npool, \
             tc.tile_pool(name="pp", bufs=8) as ppool:
            pools.update(xp=xpool, psA=psA, psV=psV, stp=stp, pyp=pyp,
                         ep=epool, rp=rpool, np=npool, pp=ppool)

            # Global unit order: qc0 of each batch kc-major (so stage-A K
            # tiles feed them incrementally); qc1-3 h-major.
            units = []
            unit_idx = {}
            for b in range(B):
                base = 64 * b
                for kc in range(4):
                    hg = ([(0, 0), (1, 0), (0, 1), (1, 1)]
                          if (b, kc) == (0, 0)
                          else [(h, g) for h in range(HC) for g in range(2)])
                    for h, g in hg:
                        unit_idx[(b, 0, h, 2 * kc + g)] = len(units)
                        units.append((b, 0, h, 2 * kc + g))
                for qc in (1, 2, 3):
                    for h in range(HC):
                        for ktg in range(NKT // 2):
                            unit_idx[(b, qc, h, ktg)] = len(units)
                            units.append((b, qc, h, ktg))

            # Lanes: ordered piece lists [(E, L, pe_cost_ns, fn), ...].
            # E = earliest slot, L = latest slot (deadline for EDF).
            lanes = []

            def a_lane(b):
                lane = []
                off = 64 * b
                if b == 0:
                    # c0's pk/pq pre-emitted before unit 0
                    lane.append((0, 14, 850, lambda: a_pv(0, 0, 0)))
                    lane.append((0, 15, 850, lambda: a_pv(0, 0, 1)))
                    for c in (1, 2, 3):
                        lane.append((0, 4 * c - 1, 1700,
                                     lambda c=c: a_pk(0, c)))
                    for c in (1, 2, 3):
                        for hf in (0, 1):
                            lane.append((1, 16, 850,
                                         lambda c=c, hf=hf: a_pv(0, c, hf)))
                    for c in (1, 2, 3):
                        lane.append((2, 16 * c - 2, 1700,
                                     lambda c=c: a_pq(0, c)))
                else:
                    E = int(os.environ.get('KV_BE', 20))
                    lane.append((E, 59, 1700, lambda: a_pk(1, 0)))
                    lane.append((E + 2, 62, 1700, lambda: a_pq(1, 0)))
                    for c in (1, 2, 3):
                        lane.append((E + 4 * c, 63 + 4 * c, 1700,
                                     lambda c=c: a_pk(1, c)))
                    for c in range(4):
                        for hf in (0, 1):
                            lane.append((E + 16 + c, 76, 850,
                                         lambda c=c, hf=hf: a_pv(1, c, hf)))
                    for c in (1, 2, 3):
                        lane.append((E + 20 + c, 63 + 16 * c, 1700,
                                     lambda c=c: a_pq(1, c)))
                return lane

            def av_lane(h):
                lane = []
                for b in range(B):
                    for qc in range(QC):
                        last = max(unit_idx[(b, qc, h, ktg)]
                                   for ktg in range(NKT // 2))
                        E = last + EXP_SLACK
                        lc = False
                        ps = av_piece_list(b, qc, h, split=lc)
                        if lc:
                            costs = (430, 430, 450, 430, 430, 450)
                            for i, p in enumerate(ps):
                                lane.append((E + i // 2, E + 5 + i, costs[i],
                                             p))
                        else:
                            lane.append((E, E + 5, 430, ps[0]))
                            lane.append((E, E + 6, 430, ps[1]))
                            lane.append((E + 1, E + 7, 430, ps[2]))
                            lane.append((E + 1, E + 8, 640, ps[3]))
                return lane

            def proj_lane():
                lane = []
                for b in range(B):
                    for qc in range(QC):
                        last = max(unit_idx[(b, qc, h, ktg)]
                                   for h in range(HC)
                                   for ktg in range(NKT // 2))
                        E = last + EXP_SLACK + int(os.environ.get('KV_PE', 12))
                        lc0 = (b == B - 1 and qc == QC - 1)
                        lc = False
                        if lc:
                            for half in range(2):
                                for ot in range(D // 128):
                                    lane.append(
                                        (E + half, E + 10 + half * 8 + ot,
                                         110,
                                         lambda ot=ot, half=half:
                                             proj_piece_half(1, 3, ot, half)))
                        else:
                            for ot in range(D // 128):
                                lane.append(
                                    (E + ot // 2, E + 10 + ot, 210,
                                     lambda b=b, qc=qc, ot=ot, lc0=lc0:
                                         proj_piece(b, qc, ot,
                                                    pool_copy=(lc0 and
                                                               ot % 2 == 1))))
                return lane

            lanes.append(a_lane(0))
            lanes.append(a_lane(1))
            lanes.append(av_lane(0))
            lanes.append(av_lane(1))
            lanes.append(proj_lane())
            heads = [0] * len(lanes)

            # PE warmup: dummy zero matmuls keep the tensor engine busy
            # through the p-state ramp while the first x chunk loads, so the
            # first real projections run at full clock.
            zeros = singles.tile([128, CW], BF16, name="zeros")
            nc.vector.memset(zeros, 0.0)
            for w in range(0):
                wps = pools["psA"].tile([128, CW], F32, tag="pqk", name="wps")
                nc.tensor.matmul(wps, lhsT=zeros[:, 0:128], rhs=zeros,
                                 start=True, stop=True)

            # head: c0 q/k before unit 0. DMA order is the critical path to
            # the first exp: stream the first two k-tiles of wq/wk, then the
            # x quarters, then the weight remainders; wv/wo/biases follow.
            xpool0 = pools["xp"]
            xt0 = xpool0.tile([128, KTD, CW], BF16, tag="xt", name="xt")
            xts[(0, 0)] = xt0
            xs0 = xT[:, 0:CW].rearrange("(ko p) n -> p ko n", p=128)
            wqr = wq.rearrange("(ko p) m -> p ko m", p=128)
            wkr = wk.rearrange("(ko p) m -> p ko m", p=128)
            nc.sync.dma_start(wq_sb[:, 0:2, :], wqr[:, 0:2, :])
            nc.sync.dma_start(wk_sb[:, 0:2, :], wkr[:, 0:2, :])
            nc.sync.dma_start(xt0[:, 0:2, :], xs0[:, 0:2, :])
            nc.sync.dma_start(xt0[:, 2:4, :], xs0[:, 2:4, :])
            nc.sync.dma_start(wq_sb[:, 2:KTD, :], wqr[:, 2:KTD, :])
            nc.sync.dma_start(wk_sb[:, 2:KTD, :], wkr[:, 2:KTD, :])
            for qq in range(2, 4):
                nc.sync.dma_start(xt0[:, 2 * qq:2 * qq + 2, :],
                                  xs0[:, 2 * qq:2 * qq + 2, :])
            nc.sync.dma_start(bq_sb, bq)
            nc.sync.dma_start(bk_sb, bk)
            nc.sync.dma_start(wv_sb, wv.rearrange("(ko p) m -> p ko m", p=128))
            nc.sync.dma_start(wo_sb, wo)
            # interleave the k-accumulations of pq0/pk0 across two banks so
            # QT/KT c0 land together ~3us sooner (first exp gates on both)
            pq0 = pools["psA"].tile([128, CW], F32, tag="pqk", name="pq")
            pk0 = pools["stp"].tile([128, 2, 512], F32, tag="st",
                                    name="pk")[:, 0, :]
            for k in range(KTD):
                nc.tensor.matmul(pq0, lhsT=wq_sb[:, k, :], rhs=xt0[:, k, :],
                                 start=(k == 0), stop=(k == KTD - 1))
                nc.tensor.matmul(pk0, lhsT=wk_sb[:, k, :], rhs=xt0[:, k, :],
                                 start=(k == 0), stop=(k == KTD - 1))
            nc.vector.tensor_tensor(QT[0][0], pq0,
                                    bq_sb.to_broadcast((128, CW)),
                                    mybir.AluOpType.add)
            # KT bias on ACT (idle before the first exp) so the two c0 bias
            # adds run in parallel instead of serializing on DVE
            nc.scalar.activation(KT[0][0], pk0, AF.Identity, bias=bk_sb)

            # EDF weave
            UNIT_COST = 430.0
            TARGET = float(os.environ.get("KV_TARGET", 900.0))
            debt = 0.0
            for idx, (b, qc, h, ktg) in enumerate(units):
                sc_unit(b, qc, h, ktg)
                budget = TARGET - UNIT_COST + debt
                while True:
                    best = None
                    urgent = False
                    for li, lane in enumerate(lanes):
                        if heads[li] >= len(lane):
                            continue
                        E, L, cost, fn = lane[heads[li]]
                        if E > idx:
                            continue
                        if L <= idx + 1:
                            best, urgent = li, True
                            break
                        if best is None or L < lanes[best][heads[best]][1]:
                            best = li
                    if best is None:
                        break
                    E, L, cost, fn = lanes[best][heads[best]]
                    if not urgent and budget < cost * float(os.environ.get('KV_BF', 0.5)):
                        break
                    fn()
                    heads[best] += 1
                    budget -= cost
                debt = min(max(budget, -2000.0), 500.0)
            # drain remaining pieces in deadline order
            rest = []
            for li, lane in enumerate(lanes):
                rest.extend(lane[heads[li]:])
            rest.sort(key=lambda p: p[1])
            for E, L, cost, fn in rest:
                fn()

    nc.compile()
    return nc


_CACHE = {}


def _get_nc():
    if "nc" not in _CACHE:
        _CACHE["nc"] = build_nc()
    return _CACHE["nc"]


def _prep_inputs(x, Wqkv, bqkv):
    bf = ml_dtypes.bfloat16
    x = np.asarray(x, np.float32)
    Wqkv = np.asarray(Wqkv, np.float32)
    bqkv = np.asarray(bqkv, np.float32)
    xT = np.ascontiguousarray(x.reshape(TOK, D).T).astype(bf)
    in_maps = []
    for c in range(NC):
        cs = slice(c * 128, (c + 1) * 128)
        in_maps.append({
            "xT": xT,
            "wq": np.ascontiguousarray(Wqkv[:, 0 * D + c * 128:0 * D + (c + 1) * 128]).astype(bf),
            "wk": np.ascontiguousarray(Wqkv[:, 1 * D + c * 128:1 * D + (c + 1) * 128]).astype(bf),
            "wv": np.ascontiguousarray(Wqkv[:, 2 * D + c * 128:2 * D + (c + 1) * 128]).astype(bf),
            "wo": None,  # filled by caller (needs Wproj)
            "bq": np.ascontiguousarray(bqkv[0 * D + c * 128:0 * D + (c + 1) * 128]).reshape(128, 1).astype(np.float32),
            "bk": np.ascontiguousarray(bqkv[1 * D + c * 128:1 * D + (c + 1) * 128]).reshape(128, 1).astype(np.float32),
        })
    return in_maps


def _run(x, Wqkv, bqkv, Wproj, bproj, trace=False):
    bf = ml_dtypes.bfloat16
    Wproj = np.asarray(Wproj, np.float32)
    bproj = np.asarray(bproj, np.float32)
    bqkv_np = np.asarray(bqkv, np.float32)
    in_maps = _prep_inputs(x, Wqkv, bqkv_np)
    for c in range(NC):
        in_maps[c]["wo"] = np.ascontiguousarray(
            Wproj[c * 128:(c + 1) * 128, :]).astype(bf)
    nc = _get_nc()
    res = run_bass_kernel_spmd(nc, in_maps, core_ids=list(range(NC)), trace=trace)
    acc = res.results[0]["out_p"].astype(np.float32).copy()
    for c in range(1, NC):
        acc += res.results[c]["out_p"]
    bv = bqkv_np[2 * D:]
    bias_eff = (bv @ Wproj + bproj).astype(np.float32)
    out = np.ascontiguousarray(acc.T).reshape(B, N, D) + bias_eff
    return out.astype(np.float32), res


def kernel(x, Wqkv, bqkv, Wproj, bproj):
    out, _ = _run(x, Wqkv, bqkv, Wproj, bproj, trace=False)
    return out



# revision 20
# speedup vs baseline: 1.0381x; 1.0381x over previous
"""Trainium2 Bass kernel for AxialMHA (B=2, N=2048, D=1024, H=16, dh=64).

Sharding: tensor-parallel over heads - 16 heads / 8 cores = 2 heads per core.
Each core computes q/k/v projections for its 2 heads (full batch), runs
attention, and produces a partial output projection (contraction over its
128 feature dims). Host sums the 8 partials and adds the effective bias
(bv @ Wproj + bproj - the v-bias commutes through softmax-weighted sums).

v3: the kernel is one global stream of 128 scores+exp "units" (ACT's exp
stream is the secondary bottleneck at ~134us busy vs PE ~137us); all other
work - stage-A projection chunks, AV accumulation groups, normalize+
transpose, output-projection columns - is chopped into sub-microsecond
"pieces" drained between units once their upstream exps are guaranteed
complete (readiness tracked by unit index, max 2 pieces per slot). This
keeps both in-order engine queues free of head-of-line blocking:
- AV consumes exp'd score tiles as the stationary operand streaming
  [V_h | ones] (65 cols) per 128-qtok group; denominators ride in col 64.
- Normalize on DVE; transpose back to feature-major via PE identity-matmul
  into psum banks time-shared with the AV accumulators.
"""

import os
import sys

import numpy as np
import ml_dtypes

for _p in ("/opt/trn_rl_repo",):
    if _p not in sys.path and os.path.isdir(_p):
        sys.path.insert(0, _p)

import concourse.bass as bass
import concourse.tile as tile
from concourse import bacc, masks, mybir
from concourse.bass_utils import run_bass_kernel_spmd

BF16 = mybir.dt.bfloat16
F32 = mybir.dt.float32
E4 = mybir.dt.float8e4
U8 = mybir.dt.uint8
AF = mybir.ActivationFunctionType
DR = mybir.MatmulPerfMode.DoubleRow
ADD = mybir.AluOpType.add
MUL = mybir.AluOpType.mult

B, N, D, H, DH = 2, 2048, 1024, 16, 64
NC = 8            # cores
HC = H // NC      # heads per core = 2
TOK = B * N       # 4096
CH = 8            # token chunks of 512 for projections
CW = TOK // CH    # 512
KTD = D // 128    # 8 contraction tiles for projections
NKT = N // 128    # 16 ktok tiles per batch
QC = N // 512     # 4 qchunks per batch
DV = DH + 1       # V cols + ones column for softmax denominators
DVP = 80          # padded V slot (DoubleRow slice stride must be %16)
# Schraudolph: uint8(A*s + B) == e4m3 bits of ~exp(s/8) (HW RNE verified)
SCH_A = float(np.log2(np.e) * 8 * 0.125)
SCH_B = 56.0 - 0.458
U_ACT = int(os.environ.get("KV_UACT", 104))  # units with exp on ACT (of 128)
EXP_SLACK = int(os.environ.get("KV_SLACK", 5))     # units of lag assumed between score emission and exp done


def build_nc():
    nc = bacc.Bacc(
        "TRN2",
        target_bir_lowering=False,
        debug=False,
        enable_asserts=False,
        num_devices=NC,
    )
    xT = nc.dram_tensor("xT", [D, TOK], BF16, kind="ExternalInput").ap()
    wq = nc.dram_tensor("wq", [D, 128], BF16, kind="ExternalInput").ap()
    wk = nc.dram_tensor("wk", [D, 128], BF16, kind="ExternalInput").ap()
    wv = nc.dram_tensor("wv", [D, 128], BF16, kind="ExternalInput").ap()
    wo = nc.dram_tensor("wo", [128, D], BF16, kind="ExternalInput").ap()
    bq = nc.dram_tensor("bq", [128, 1], F32, kind="ExternalInput").ap()
    bk = nc.dram_tensor("bk", [128, 1], F32, kind="ExternalInput").ap()
    out_p = nc.dram_tensor("out_p", [D, TOK], BF16, kind="ExternalOutput").ap()

    from contextlib import ExitStack

    with tile.TileContext(nc) as tc, ExitStack() as ctx:
        singles = ctx.enter_context(tc.tile_pool(name="singles", bufs=1))

        bq_sb = singles.tile([128, 1], F32)
        bk_sb = singles.tile([128, 1], F32)
        wq_sb = singles.tile([128, KTD, 128], BF16)
        wk_sb = singles.tile([128, KTD, 128], BF16)
        wv_sb = singles.tile([128, KTD, 128], BF16)
        wo_sb = singles.tile([128, D], BF16)
        identity = singles.tile([128, 128], BF16)
        masks.make_identity(nc, identity[:])
        warm_i = singles.tile([128, 1], F32)
        warm_o = singles.tile([128, 1], E4)

        QT = [[singles.tile([128, CW], BF16, name=f"QT{b}_{q}") for q in range(QC)]
              for b in range(B)]
        KT = [[singles.tile([128, CW], BF16, name=f"KT{b}_{q}") for q in range(QC)]
              for b in range(B)]
        V1 = [[singles.tile([128, 4, HC, DVP], E4, name=f"V1{b}_{q}")
               for q in range(QC)] for b in range(B)]
        for b in range(B):
            for q in range(QC):
                nc.vector.memset(V1[b][q], 1.0)
        yT = [singles.tile([128, N], BF16, name=f"yT{b}") for b in range(B)]

        ets = {}  # (b, qc, h, ktg) -> et tile (e4m3)
        pools = {}
        exp_on_act = {}

        # ---- emission helpers -------------------------------------------
        def chunk_qk(b, cc, first=False):
            xpool, psA = pools["xp"], pools["psA"]
            c = b * (CH // B) + cc
            xt = xpool.tile([128, KTD, CW], BF16, tag="xt", name="xt")
            xs = xT[:, c * CW:(c + 1) * CW].rearrange("(ko p) n -> p ko n", p=128)
            if first:
                for qq in range(4):
                    nc.sync.dma_start(xt[:, 2 * qq:2 * qq + 2, :],
                                      xs[:, 2 * qq:2 * qq + 2, :])
                nc.sync.dma_start(wk_sb, wk.rearrange("(ko p) m -> p ko m", p=128))
                nc.sync.dma_start(wv_sb, wv.rearrange("(ko p) m -> p ko m", p=128))
            else:
                nc.sync.dma_start(xt[:, 0:KTD // 2, :], xs[:, 0:KTD // 2, :])
                nc.sync.dma_start(xt[:, KTD // 2:, :], xs[:, KTD // 2:, :])
            pq = psA.tile([128, CW], F32, tag="pqk", name="pq")
            for k in range(KTD):
                nc.tensor.matmul(pq, lhsT=wq_sb[:, k, :], rhs=xt[:, k, :],
                                 start=(k == 0), stop=(k == KTD - 1))
            nc.vector.tensor_tensor(QT[b][cc], pq,
                                    bq_sb.to_broadcast((128, CW)),
                                    mybir.AluOpType.add)
            pk = psA.tile([128, CW], F32, tag="pqk", name="pk")
            for k in range(KTD):
                nc.tensor.matmul(pk, lhsT=wk_sb[:, k, :], rhs=xt[:, k, :],
                                 start=(k == 0), stop=(k == KTD - 1))
            nc.vector.tensor_tensor(KT[b][cc], pk,
                                    bk_sb.to_broadcast((128, CW)),
                                    mybir.AluOpType.add)
            return xt

        def chunk_v(b, cc, xt):
            psV = pools["psV"]
            pv = psV.tile([128, 4, 128], F32, tag="pp", name="pv")
            for s in range(CW // 128):
                for k in range(KTD):
                    nc.tensor.matmul(pv[:, s, :],
                                     lhsT=xt[:, k, s * 128:(s + 1) * 128],
                                     rhs=wv_sb[:, k, :],
                                     start=(k == 0), stop=(k == KTD - 1))
            for h in range(HC):
                nc.vector.tensor_copy(V1[b][cc][:, :, h, 0:DH],
                                      pv[:, :, h * DH:(h + 1) * DH])

        def sc_unit(idx, b, qc, h, ktg):
            stp, epool = pools["stp"], pools["ep"]
            hs = slice(h * DH, (h + 1) * DH)
            stt = stp.tile([128, 2, 512], F32, tag="st", name="stt")
            for j in range(2):
                kt = ktg * 2 + j
                kc, ks = divmod(kt, 4)
                nc.tensor.matmul(
                    stt[:, j, :],
                    lhsT=KT[b][kc][hs, ks * 128:(ks + 1) * 128],
                    rhs=QT[b][qc][hs, :],
                    start=True, stop=True,
                    tile_position=(h * DH, 0),
                )
            et = epool.tile([128, 2, 512], E4, tag="et", name="et")
            if exp_on_act.get(idx, True):
                nc.scalar.activation(et, stt, AF.Exp, scale=0.125)
            else:
                nc.vector.tensor_scalar(et.bitcast(U8), stt,
                                        SCH_A, SCH_B, MUL, ADD)
            ets[(b, qc, h, ktg)] = et

        def av_group(b, qc, h, qt, py):
            # DoubleRow: each instr contracts one et tile's 2 ktok slices
            for ktg in range(NKT // 2):
                kc, j = divmod(ktg, 2)
                nc.tensor.matmul(
                    py[:, qt, 0:DV],
                    lhsT=ets[(b, qc, h, ktg)][:, :, qt * 128:(qt + 1) * 128],
                    rhs=V1[b][kc][:, 2 * j:2 * j + 2, h, 0:DV],
                    start=(ktg == 0), stop=(ktg == NKT // 2 - 1),
                    perf_mode=DR)

        def av_norm(b, qc, h, py):
            rpool, npool, pyp = pools["rp"], pools["np"], pools["pyp"]
            qo = qc * 512
            rsb = rpool.tile([128, 4, 1], F32, tag="rsb", name="rsb")
            nc.vector.reciprocal(rsb, py[:, :, DH:DV])
            yn = npool.tile([128, 4, DH], BF16, tag="yn", name="yn")
            nc.vector.tensor_tensor(yn, py[:, :, 0:DH],
                                    rsb.to_broadcast((128, 4, DH)),
                                    mybir.AluOpType.mult)
            ytr = pyp.tile([128, 4, 128], F32, tag=f"py{h}", name=f"ytr{h}")
            for qt in range(4):
                # transpose as a plain matmul: yn.T @ I  ->  [64 x 128]
                nc.tensor.matmul(ytr[0:DH, qt, :], lhsT=yn[:, qt, :],
                                 rhs=identity, start=True, stop=True)
            nc.vector.tensor_copy(yT[b][h * DH:(h + 1) * DH, qo:qo + 512],
                                  ytr[0:DH, 0:4, :])

        def av_norm_half(b, qc, h, py, half, state):
            # last chunk: normalize/transpose/copy one 256-token half so the
            # output projection and stores start before the other half's AV
            rpool, npool = pools["rp"], pools["np"]
            qo = qc * 512
            q0 = 2 * half
            rsb = rpool.tile([128, 2, 1], F32, tag="rs2", name="rs2")
            nc.vector.reciprocal(rsb, py[:, q0:q0 + 2, DH:DV])
            yn = npool.tile([128, 2, DH], BF16, tag="yn2", name="yn2")
            nc.vector.tensor_tensor(yn, py[:, q0:q0 + 2, 0:DH],
                                    rsb.to_broadcast((128, 2, DH)),
                                    mybir.AluOpType.mult)
            if half == 0:
                state["ytr"] = pools["stp"].tile([128, 8, 128], F32,
                                                 tag="st", name="ytr_l")
            ytr = state["ytr"]
            for qt in (q0, q0 + 1):
                nc.tensor.matmul(ytr[0:DH, qt, :], lhsT=yn[:, qt - q0, :],
                                 rhs=identity, start=True, stop=True)
            nc.vector.tensor_copy(
                yT[b][h * DH:(h + 1) * DH, qo + half * 256:qo + half * 256 + 256],
                ytr[0:DH, q0:q0 + 2, :])

        def av_piece_list(b, qc, h, split=False):
            state = {}

            def alloc_g0():
                state["py"] = pools["pyp"].tile(
                    [128, 4, 128], F32, tag=f"py{h}", name=f"py{h}")
                av_group(b, qc, h, 0, state["py"])

            if split:
                return [
                    alloc_g0,
                    lambda: av_group(b, qc, h, 1, state["py"]),
                    lambda: av_norm_half(b, qc, h, state["py"], 0, state),
                    lambda: av_group(b, qc, h, 2, state["py"]),
                    lambda: av_group(b, qc, h, 3, state["py"]),
                    lambda: av_norm_half(b, qc, h, state["py"], 1, state),
                ]
            return [
                alloc_g0,
                lambda: av_group(b, qc, h, 1, state["py"]),
                lambda: av_group(b, qc, h, 2, state["py"]),
                lambda: (av_group(b, qc, h, 3, state["py"]),
                         av_norm(b, qc, h, state["py"])),
            ]

        proj_ps = {}
        proj_pp = {}

        def proj_piece(b, cc, ot, pool_copy=False, wide=False):
            # ot pairs (0,1), (2,3), ... share one staging tile and one DMA
            cs = slice(cc * CW, (cc + 1) * CW)
            if ot % 2 == 0:
                pp = pools["psV"].tile([128, CW], F32, tag="pp", name="pp")
            else:
                pp = pools["psA"].tile([128, CW], F32, tag="pqk", name="pp")
            nc.tensor.matmul(pp, lhsT=wo_sb[:, ot * 128:(ot + 1) * 128],
                             rhs=yT[b][:, cs],
                             start=True, stop=True)
            if ot % 2 == 0:
                ps = pools["pp"].tile([128, 2, CW], BF16, tag="ps", name="ps")
                proj_ps[(b, cc)] = ps
            else:
                ps = proj_ps[(b, cc)]
            if pool_copy:
                nc.scalar.copy(ps[:, ot % 2, :], pp)
            else:
                nc.vector.tensor_copy(ps[:, ot % 2, :], pp)
            if ot % 2 == 1:
                po = ot - 1
                dst = out_p[po * 128:(po + 2) * 128,
                            b * N + cc * CW:b * N + (cc + 1) * CW]
                nc.sync.dma_start(dst.rearrange("(o p) n -> p o n", p=128), ps)

        def proj_piece_half(b, cc, ot, half):
            # last chunk: 256-token halves, stores fire per (pair, half)
            ts = slice(cc * CW + half * 256, cc * CW + half * 256 + 256)
            if ot % 2 == 0:
                pp = pools["psV"].tile([128, CW], F32, tag="pp", name="pp")
            else:
                pp = pools["psA"].tile([128, CW], F32, tag="pqk", name="pp")
            nc.tensor.matmul(pp[:, 0:256],
                             lhsT=wo_sb[:, ot * 128:(ot + 1) * 128],
                             rhs=yT[b][:, ts],
                             start=True, stop=True)
            if ot % 2 == 0 and half == 0:
                ps = pools["pp"].tile([128, 2, CW], BF16, tag="ps", name="ps")
                proj_ps[(b, cc, ot // 2)] = ps
            else:
                ps = proj_ps[(b, cc, ot // 2)]
            if ot % 2 == 1:
                nc.scalar.copy(ps[:, 1, half * 256:half * 256 + 256],
                               pp[:, 0:256])
            else:
                nc.vector.tensor_copy(ps[:, 0, half * 256:half * 256 + 256],
                                      pp[:, 0:256])
            if ot % 2 == 1:
                po = ot - 1
                dst = out_p[po * 128:(po + 2) * 128, b * N + ts.start:
                            b * N + ts.stop]
                nc.sync.dma_start(dst.rearrange("(o p) n -> p o n", p=128),
                                  ps[:, :, half * 256:half * 256 + 256])

        # ---- split stage-A chunk emitters for lane scheduling -----------
        xts = {}

        def a_pk(b, c, first=False):
            xpool, psA = pools["xp"], pools["psA"]
            cg = b * (CH // B) + c
            xt = xpool.tile([128, KTD, CW], BF16, tag="xt", name="xt")
            xts[(b, c)] = xt
            xs = xT[:, cg * CW:(cg + 1) * CW].rearrange(
                "(ko p) n -> p ko n", p=128)
            nc.sync.dma_start(xt[:, 0:KTD // 2, :], xs[:, 0:KTD // 2, :])
            nc.sync.dma_start(xt[:, KTD // 2:, :], xs[:, KTD // 2:, :])
            pk = psA.tile([128, CW], F32, tag="pqk", name="pk")
            for k in range(KTD):
                nc.tensor.matmul(pk, lhsT=wk_sb[:, k, :], rhs=xt[:, k, :],
                                 start=(k == 0), stop=(k == KTD - 1))
            nc.vector.tensor_tensor(KT[b][c], pk,
                                    bk_sb.to_broadcast((128, CW)),
                                    mybir.AluOpType.add)

        def a_pq(b, c):
            psA = pools["psA"]
            xt = xts[(b, c)]
            pq = psA.tile([128, CW], F32, tag="pqk", name="pq")
            for k in range(KTD):
                nc.tensor.matmul(pq, lhsT=wq_sb[:, k, :], rhs=xt[:, k, :],
                                 start=(k == 0), stop=(k == KTD - 1))
            nc.vector.tensor_tensor(QT[b][c], pq,
                                    bq_sb.to_broadcast((128, CW)),
                                    mybir.AluOpType.add)

        def a_pv(b, c, half):
            psV = pools["psV"]
            xt = xts[(b, c)]
            pv = psV.tile([128, 2, 128], F32, tag="pp", name="pv")
            for si, s in enumerate((2 * half, 2 * half + 1)):
                for k in range(KTD):
                    nc.tensor.matmul(pv[:, si, :],
                                     lhsT=xt[:, k, s * 128:(s + 1) * 128],
                                     rhs=wv_sb[:, k, :],
                                     start=(k == 0), stop=(k == KTD - 1))
            for h in range(HC):
                nc.vector.tensor_copy(
                    V1[b][c][:, 2 * half:2 * half + 2, h, 0:DH],
                    pv[:, :, h * DH:(h + 1) * DH])

        # ---- full emission: EDF-woven unit/piece stream -----------------
        with tc.tile_pool(name="xp", bufs=4) as xpool, \
             tc.tile_pool(name="psA", bufs=1, space="PSUM") as psA, \
             tc.tile_pool(name="psV", bufs=1, space="PSUM") as psV, \
             tc.tile_pool(name="stp", bufs=2, space="PSUM") as stp, \
             tc.tile_pool(name="pyp", bufs=1, space="PSUM") as pyp, \
             tc.tile_pool(name="ep", bufs=int(os.environ.get("KV_EP", 46))) as epool, \
             tc.tile_pool(name="rp", bufs=4) as rpool, \
             tc.tile_pool(name="np", bufs=4) as npool, \
             tc.tile_pool(name="pp", bufs=8) as ppool:
            pools.update(xp=xpool, psA=psA, psV=psV, stp=stp, pyp=pyp,
                         ep=epool, rp=rpool, np=npool, pp=ppool)

            # Global unit order: qc0 of each batch kc-major (so stage-A K
            # tiles feed them incrementally); qc1-3 h-major.
            units = []
            unit_idx = {}
            for b in range(B):
                base = 64 * b
                for kc in range(4):
                    hg = ([(0, 0), (1, 0), (0, 1), (1, 1)]
                          if (b, kc) == (0, 0)
                          else [(h, g) for h in range(HC) for g in range(2)])
                    for h, g in hg:
                        unit_idx[(b, 0, h, 2 * kc + g)] = len(units)
                        units.append((b, 0, h, 2 * kc + g))
                for qc in (1, 2, 3):
                    for h in range(HC):
                        for ktg in range(NKT // 2):
                            unit_idx[(b, qc, h, ktg)] = len(units)
                            units.append((b, qc, h, ktg))

            # warmup exp on a dedicated tiny tile: hoists the ACT table
            # load off the first real exp's critical path
            nc.vector.memset(warm_i, 0.0)
            nc.scalar.activation(warm_o, warm_i, AF.Exp)

            # exp engine split: U_ACT units on ACT spread evenly, rest DVE
            n_dve = 128 - U_ACT
            acc = 0.0
            for i in range(128):
                acc += n_dve / 128.0
                if acc >= 1.0:
                    acc -= 1.0
                    exp_on_act[i] = False
                else:
                    exp_on_act[i] = True

            # Lanes: ordered piece lists [(E, L, pe_cost_ns, fn), ...].
            # E = earliest slot, L = latest slot (deadline for EDF).
            lanes = []

            def a_lane(b):
                lane = []
                off = 64 * b
                if b == 0:
                    # c0's pk/pq pre-emitted before unit 0
                    lane.append((0, 14, 850, lambda: a_pv(0, 0, 0)))
                    lane.append((0, 15, 850, lambda: a_pv(0, 0, 1)))
                    for c in (1, 2, 3):
                        lane.append((0, 4 * c - 1, 1700,
                                     lambda c=c: a_pk(0, c)))
                    for c in (1, 2, 3):
                        for hf in (0, 1):
                            lane.append((1, 16, 850,
                                         lambda c=c, hf=hf: a_pv(0, c, hf)))
                    for c in (1, 2, 3):
                        lane.append((2, 16 * c - 2, 1700,
                                     lambda c=c: a_pq(0, c)))
                else:
                    E = int(os.environ.get('KV_BE', 20))
                    lane.append((E, 59, 1700, lambda: a_pk(1, 0)))
                    lane.append((E + 2, 62, 1700, lambda: a_pq(1, 0)))
                    for c in (1, 2, 3):
                        lane.append((E + 4 * c, 63 + 4 * c, 1700,
                                     lambda c=c: a_pk(1, c)))
                    for c in range(4):
                        for hf in (0, 1):
                            lane.append((E + 16 + c, 76, 850,
                                         lambda c=c, hf=hf: a_pv(1, c, hf)))
                    for c in (1, 2, 3):
                        lane.append((E + 20 + c, 63 + 16 * c, 1700,
                                     lambda c=c: a_pq(1, c)))
                return lane

            def av_lane(h):
                lane = []
                for b in range(B):
                    for qc in range(QC):
                        last = max(unit_idx[(b, qc, h, ktg)]
                                   for ktg in range(NKT // 2))
                        E = last + EXP_SLACK
                        lc = False
                        ps = av_piece_list(b, qc, h, split=lc)
                        if lc:
                            costs = (430, 430, 450, 430, 430, 450)
                            for i, p in enumerate(ps):
                                lane.append((E + i // 2, E + 5 + i, costs[i],
                                             p))
                        else:
                            lane.append((E, E + 5, 120, ps[0]))
                            lane.append((E, E + 6, 120, ps[1]))
                            lane.append((E + 1, E + 7, 120, ps[2]))
                            lane.append((E + 1, E + 8, 430, ps[3]))
                return lane

            def proj_lane():
                lane = []
                for b in range(B):
                    for qc in range(QC):
                        last = max(unit_idx[(b, qc, h, ktg)]
                                   for h in range(HC)
                                   for ktg in range(NKT // 2))
                        E = last + EXP_SLACK + int(os.environ.get('KV_PE', 12))
                        lc0 = (b == B - 1 and qc == QC - 1)
                        lc = False
                        if lc:
                            for half in range(2):
                                for ot in range(D // 128):
                                    lane.append(
                                        (E + half, E + 10 + half * 8 + ot,
                                         110,
                                         lambda ot=ot, half=half:
                                             proj_piece_half(1, 3, ot, half)))
                        else:
                            for ot in range(D // 128):
                                lane.append(
                                    (E + ot // 2, E + 10 + ot, 210,
                                     lambda b=b, qc=qc, ot=ot, lc0=lc0:
                                         proj_piece(b, qc, ot,
                                                    pool_copy=(lc0 and
                                                               ot % 2 == 1))))
                return lane

            lanes.append(a_lane(0))
            lanes.append(a_lane(1))
            lanes.append(av_lane(0))
            lanes.append(av_lane(1))
            lanes.append(proj_lane())
            heads = [0] * len(lanes)

            # PE warmup: dummy zero matmuls keep the tensor engine busy
            # through the p-state ramp while the first x chunk loads, so the
            # first real projections run at full clock.
            zeros = singles.tile([128, CW], BF16, name="zeros")
            nc.vector.memset(zeros, 0.0)
            for w in range(0):
                wps = pools["psA"].tile([128, CW], F32, tag="pqk", name="wps")
                nc.tensor.matmul(wps, lhsT=zeros[:, 0:128], rhs=zeros,
                                 start=True, stop=True)

            # head: c0 q/k before unit 0. DMA order is the critical path to
            # the first exp: stream the first two k-tiles of wq/wk, then the
            # x quarters, then the weight remainders; wv/wo/biases follow.
            xpool0 = pools["xp"]
            xt0 = xpool0.tile([128, KTD, CW], BF16, tag="xt", name="xt")
            xts[(0, 0)] = xt0
            xs0 = xT[:, 0:CW].rearrange("(ko p) n -> p ko n", p=128)
            wqr = wq.rearrange("(ko p) m -> p ko m", p=128)
            wkr = wk.rearrange("(ko p) m -> p ko m", p=128)
            nc.sync.dma_start(wq_sb[:, 0:2, :], wqr[:, 0:2, :])
            nc.sync.dma_start(wk_sb[:, 0:2, :], wkr[:, 0:2, :])
            nc.sync.dma_start(xt0[:, 0:2, :], xs0[:, 0:2, :])
            nc.sync.dma_start(xt0[:, 2:4, :], xs0[:, 2:4, :])
            nc.sync.dma_start(wq_sb[:, 2:KTD, :], wqr[:, 2:KTD, :])
            nc.sync.dma_start(wk_sb[:, 2:KTD, :], wkr[:, 2:KTD, :])
            for qq in range(2, 4):
                nc.sync.dma_start(xt0[:, 2 * qq:2 * qq + 2, :],
                                  xs0[:, 2 * qq:2 * qq + 2, :])
            nc.sync.dma_start(bq_sb, bq)
            nc.sync.dma_start(bk_sb, bk)
            nc.sync.dma_start(wv_sb, wv.rearrange("(ko p) m -> p ko m", p=128))
            nc.sync.dma_start(wo_sb, wo)
            # interleave the k-accumulations of pq0/pk0 across two banks so
            # QT/KT c0 land together ~3us sooner (first exp gates on both)
            pq0 = pools["psA"].tile([128, CW], F32, tag="pqk", name="pq")
            pk0 = pools["stp"].tile([128, 2, 512], F32, tag="st",
                                    name="pk")[:, 0, :]
            for k in range(KTD):
                nc.tensor.matmul(pq0, lhsT=wq_sb[:, k, :], rhs=xt0[:, k, :],
                                 start=(k == 0), stop=(k == KTD - 1))
                nc.tensor.matmul(pk0, lhsT=wk_sb[:, k, :], rhs=xt0[:, k, :],
                                 start=(k == 0), stop=(k == KTD - 1))
            nc.vector.tensor_tensor(QT[0][0], pq0,
                                    bq_sb.to_broadcast((128, CW)),
                                    mybir.AluOpType.add)
            # KT bias on ACT (idle before the first exp) so the two c0 bias
            # adds run in parallel instead of serializing on DVE
            nc.scalar.activation(KT[0][0], pk0, AF.Identity, bias=bk_sb)

            # EDF weave
            UNIT_COST = 430.0
            TARGET = float(os.environ.get("KV_TARGET", 780.0))
            debt = 0.0
            for idx, (b, qc, h, ktg) in enumerate(units):
                sc_unit(idx, b, qc, h, ktg)
                budget = TARGET - UNIT_COST + debt
                while True:
                    best = None
                    urgent = False
                    for li, lane in enumerate(lanes):
                        if heads[li] >= len(lane):
                            continue
                        E, L, cost, fn = lane[heads[li]]
                        if E > idx:
                            continue
                        if L <= idx + 1:
                            best, urgent = li, True
                            break
                        if best is None or L < lanes[best][heads[best]][1]:
                            best = li
                    if best is None:
                        break
                    E, L, cost, fn = lanes[best][heads[best]]
                    if not urgent and budget < cost * float(os.environ.get('KV_BF', 0.5)):
                        break
                    fn()
                    heads[best] += 1
                    budget -= cost
                debt = min(max(budget, -2000.0), 500.0)
            # drain remaining pieces in deadline order
            rest = []
            for li, lane in enumerate(lanes):
                rest.extend(lane[heads[li]:])
            rest.sort(key=lambda p: p[1])
            for E, L, cost, fn in rest:
                fn()

    nc.compile()
    return nc


_CACHE = {}


def _get_nc():
    if "nc" not in _CACHE:
        _CACHE["nc"] = build_nc()
    return _CACHE["nc"]


def _prep_inputs(x, Wqkv, bqkv):
    bf = ml_dtypes.bfloat16
    x = np.asarray(x, np.float32)
    Wqkv = np.asarray(Wqkv, np.float32)
    bqkv = np.asarray(bqkv, np.float32)
    xT = np.ascontiguousarray(x.reshape(TOK, D).T).astype(bf)
    in_maps = []
    for c in range(NC):
        cs = slice(c * 128, (c + 1) * 128)
        in_maps.append({
            "xT": xT,
            "wq": np.ascontiguousarray(Wqkv[:, 0 * D + c * 128:0 * D + (c + 1) * 128]).astype(bf),
            "wk": np.ascontiguousarray(Wqkv[:, 1 * D + c * 128:1 * D + (c + 1) * 128]).astype(bf),
            "wv": np.ascontiguousarray(Wqkv[:, 2 * D + c * 128:2 * D + (c + 1) * 128]).astype(bf),
            "wo": None,  # filled by caller (needs Wproj)
            "bq": np.ascontiguousarray(bqkv[0 * D + c * 128:0 * D + (c + 1) * 128]).reshape(128, 1).astype(np.float32),
            "bk": np.ascontiguousarray(bqkv[1 * D + c * 128:1 * D + (c + 1) * 128]).reshape(128, 1).astype(np.float32),
        })
    return in_maps


def _run(x, Wqkv, bqkv, Wproj, bproj, trace=False):
    bf = ml_dtypes.bfloat16
    Wproj = np.asarray(Wproj, np.float32)
    bproj = np.asarray(bproj, np.float32)
    bqkv_np = np.asarray(bqkv, np.float32)
    in_maps = _prep_inputs(x, Wqkv, bqkv_np)
    for c in range(NC):
        in_maps[c]["wo"] = np.ascontiguousarray(
            Wproj[c * 128:(c + 1) * 128, :]).astype(bf)
    nc = _get_nc()
    res = run_bass_kernel_spmd(nc, in_maps, core_ids=list(range(NC)), trace=trace)
    acc = res.results[0]["out_p"].astype(np.float32).copy()
    for c in range(1, NC):
        acc += res.results[c]["out_p"]
    bv = bqkv_np[2 * D:]
    bias_eff = (bv @ Wproj + bproj).astype(np.float32)
    out = np.ascontiguousarray(acc.T).reshape(B, N, D) + bias_eff
    return out.astype(np.float32), res


def kernel(x, Wqkv, bqkv, Wproj, bproj):
    out, _ = _run(x, Wqkv, bqkv, Wproj, bproj, trace=False)
    return out



# revision 21
# speedup vs baseline: 1.0413x; 1.0031x over previous
"""Trainium2 Bass kernel for AxialMHA (B=2, N=2048, D=1024, H=16, dh=64).

Sharding: tensor-parallel over heads - 16 heads / 8 cores = 2 heads per core.
Each core computes q/k/v projections for its 2 heads (full batch), runs
attention, and produces a partial output projection (contraction over its
128 feature dims). Host sums the 8 partials and adds the effective bias
(bv @ Wproj + bproj - the v-bias commutes through softmax-weighted sums).

v3: the kernel is one global stream of 128 scores+exp "units" (ACT's exp
stream is the secondary bottleneck at ~134us busy vs PE ~137us); all other
work - stage-A projection chunks, AV accumulation groups, normalize+
transpose, output-projection columns - is chopped into sub-microsecond
"pieces" drained between units once their upstream exps are guaranteed
complete (readiness tracked by unit index, max 2 pieces per slot). This
keeps both in-order engine queues free of head-of-line blocking:
- AV consumes exp'd score tiles as the stationary operand streaming
  [V_h | ones] (65 cols) per 128-qtok group; denominators ride in col 64.
- Normalize on DVE; transpose back to feature-major via PE identity-matmul
  into psum banks time-shared with the AV accumulators.
"""

import os
import sys

import numpy as np
import ml_dtypes

for _p in ("/opt/trn_rl_repo",):
    if _p not in sys.path and os.path.isdir(_p):
        sys.path.insert(0, _p)

import concourse.bass as bass
import concourse.tile as tile
from concourse import bacc, masks, mybir
from concourse.bass_utils import run_bass_kernel_spmd

BF16 = mybir.dt.bfloat16
F32 = mybir.dt.float32
E4 = mybir.dt.float8e4
U8 = mybir.dt.uint8
AF = mybir.ActivationFunctionType
DR = mybir.MatmulPerfMode.DoubleRow
ADD = mybir.AluOpType.add
MUL = mybir.AluOpType.mult

B, N, D, H, DH = 2, 2048, 1024, 16, 64
NC = 8            # cores
HC = H // NC      # heads per core = 2
TOK = B * N       # 4096
CH = 8            # token chunks of 512 for projections
CW = TOK // CH    # 512
KTD = D // 128    # 8 contraction tiles for projections
NKT = N // 128    # 16 ktok tiles per batch
QC = N // 512     # 4 qchunks per batch
DV = DH + 1       # V cols + ones column for softmax denominators
DVP = 80          # padded V slot (DoubleRow slice stride must be %16)
# Schraudolph: uint8(A*s + B) == e4m3 bits of ~exp(s/8) (HW RNE verified)
SCH_A = float(np.log2(np.e) * 8 * 0.125)
SCH_B = 56.0 - 0.458
U_ACT = int(os.environ.get("KV_UACT", 100))  # units with exp on ACT (of 128)
EXP_SLACK = int(os.environ.get("KV_SLACK", 5))     # units of lag assumed between score emission and exp done


def build_nc():
    nc = bacc.Bacc(
        "TRN2",
        target_bir_lowering=False,
        debug=False,
        enable_asserts=False,
        num_devices=NC,
    )
    xT = nc.dram_tensor("xT", [D, TOK], BF16, kind="ExternalInput").ap()
    wq = nc.dram_tensor("wq", [D, 128], BF16, kind="ExternalInput").ap()
    wk = nc.dram_tensor("wk", [D, 128], BF16, kind="ExternalInput").ap()
    wv = nc.dram_tensor("wv", [D, 128], BF16, kind="ExternalInput").ap()
    wo = nc.dram_tensor("wo", [128, D], BF16, kind="ExternalInput").ap()
    bq = nc.dram_tensor("bq", [128, 1], F32, kind="ExternalInput").ap()
    bk = nc.dram_tensor("bk", [128, 1], F32, kind="ExternalInput").ap()
    out_p = nc.dram_tensor("out_p", [D, TOK], BF16, kind="ExternalOutput").ap()

    from contextlib import ExitStack

    with tile.TileContext(nc) as tc, ExitStack() as ctx:
        singles = ctx.enter_context(tc.tile_pool(name="singles", bufs=1))

        bq_sb = singles.tile([128, 1], F32)
        bk_sb = singles.tile([128, 1], F32)
        wq_sb = singles.tile([128, KTD, 128], BF16)
        wk_sb = singles.tile([128, KTD, 128], BF16)
        wv_sb = singles.tile([128, KTD, 128], BF16)
        wo_sb = singles.tile([128, D], BF16)
        identity = singles.tile([128, 128], BF16)
        masks.make_identity(nc, identity[:])
        warm_i = singles.tile([128, 1], F32)
        warm_o = singles.tile([128, 1], E4)

        QT = [[singles.tile([128, CW], BF16, name=f"QT{b}_{q}") for q in range(QC)]
              for b in range(B)]
        KT = [[singles.tile([128, CW], BF16, name=f"KT{b}_{q}") for q in range(QC)]
              for b in range(B)]
        V1 = [[singles.tile([128, 4, HC, DVP], E4, name=f"V1{b}_{q}")
               for q in range(QC)] for b in range(B)]
        for b in range(B):
            for q in range(QC):
                nc.vector.memset(V1[b][q], 1.0)
        yT = [singles.tile([128, N], BF16, name=f"yT{b}") for b in range(B)]

        ets = {}  # (b, qc, h, ktg) -> et tile (e4m3)
        pools = {}
        exp_on_act = {}

        # ---- emission helpers -------------------------------------------
        def chunk_qk(b, cc, first=False):
            xpool, psA = pools["xp"], pools["psA"]
            c = b * (CH // B) + cc
            xt = xpool.tile([128, KTD, CW], BF16, tag="xt", name="xt")
            xs = xT[:, c * CW:(c + 1) * CW].rearrange("(ko p) n -> p ko n", p=128)
            if first:
                for qq in range(4):
                    nc.sync.dma_start(xt[:, 2 * qq:2 * qq + 2, :],
                                      xs[:, 2 * qq:2 * qq + 2, :])
                nc.sync.dma_start(wk_sb, wk.rearrange("(ko p) m -> p ko m", p=128))
                nc.sync.dma_start(wv_sb, wv.rearrange("(ko p) m -> p ko m", p=128))
            else:
                nc.sync.dma_start(xt[:, 0:KTD // 2, :], xs[:, 0:KTD // 2, :])
                nc.sync.dma_start(xt[:, KTD // 2:, :], xs[:, KTD // 2:, :])
            pq = psA.tile([128, CW], F32, tag="pqk", name="pq")
            for k in range(KTD):
                nc.tensor.matmul(pq, lhsT=wq_sb[:, k, :], rhs=xt[:, k, :],
                                 start=(k == 0), stop=(k == KTD - 1))
            nc.vector.tensor_tensor(QT[b][cc], pq,
                                    bq_sb.to_broadcast((128, CW)),
                                    mybir.AluOpType.add)
            pk = psA.tile([128, CW], F32, tag="pqk", name="pk")
            for k in range(KTD):
                nc.tensor.matmul(pk, lhsT=wk_sb[:, k, :], rhs=xt[:, k, :],
                                 start=(k == 0), stop=(k == KTD - 1))
            nc.vector.tensor_tensor(KT[b][cc], pk,
                                    bk_sb.to_broadcast((128, CW)),
                                    mybir.AluOpType.add)
            return xt

        def chunk_v(b, cc, xt):
            psV = pools["psV"]
            pv = psV.tile([128, 4, 128], F32, tag="pp", name="pv")
            for s in range(CW // 128):
                for k in range(KTD):
                    nc.tensor.matmul(pv[:, s, :],
                                     lhsT=xt[:, k, s * 128:(s + 1) * 128],
                                     rhs=wv_sb[:, k, :],
                                     start=(k == 0), stop=(k == KTD - 1))
            for h in range(HC):
                nc.vector.tensor_copy(V1[b][cc][:, :, h, 0:DH],
                                      pv[:, :, h * DH:(h + 1) * DH])

        def sc_unit(idx, b, qc, h, ktg):
            stp, epool = pools["stp"], pools["ep"]
            hs = slice(h * DH, (h + 1) * DH)
            stt = stp.tile([128, 2, 512], F32, tag="st", name="stt")
            for j in range(2):
                kt = ktg * 2 + j
                kc, ks = divmod(kt, 4)
                nc.tensor.matmul(
                    stt[:, j, :],
                    lhsT=KT[b][kc][hs, ks * 128:(ks + 1) * 128],
                    rhs=QT[b][qc][hs, :],
                    start=True, stop=True,
                    tile_position=(h * DH, 0),
                )
            et = epool.tile([128, 2, 512], E4, tag="et", name="et")
            if exp_on_act.get(idx, True):
                nc.scalar.activation(et, stt, AF.Exp, scale=0.125)
            else:
                nc.vector.tensor_scalar(et.bitcast(U8), stt,
                                        SCH_A, SCH_B, MUL, ADD)
            ets[(b, qc, h, ktg)] = et

        def av_group(b, qc, h, qt, py):
            # DoubleRow: each instr contracts one et tile's 2 ktok slices
            for ktg in range(NKT // 2):
                kc, j = divmod(ktg, 2)
                nc.tensor.matmul(
                    py[:, qt, 0:DV],
                    lhsT=ets[(b, qc, h, ktg)][:, :, qt * 128:(qt + 1) * 128],
                    rhs=V1[b][kc][:, 2 * j:2 * j + 2, h, 0:DV],
                    start=(ktg == 0), stop=(ktg == NKT // 2 - 1),
                    perf_mode=DR)

        def av_norm(b, qc, h, py):
            rpool, npool, pyp = pools["rp"], pools["np"], pools["pyp"]
            qo = qc * 512
            rsb = rpool.tile([128, 4, 1], F32, tag="rsb", name="rsb")
            nc.vector.reciprocal(rsb, py[:, :, DH:DV])
            yn = npool.tile([128, 4, DH], BF16, tag="yn", name="yn")
            nc.vector.tensor_tensor(yn, py[:, :, 0:DH],
                                    rsb.to_broadcast((128, 4, DH)),
                                    mybir.AluOpType.mult)
            ytr = pyp.tile([128, 4, 128], F32, tag=f"py{h}", name=f"ytr{h}")
            for qt in range(4):
                # transpose as a plain matmul: yn.T @ I  ->  [64 x 128]
                nc.tensor.matmul(ytr[0:DH, qt, :], lhsT=yn[:, qt, :],
                                 rhs=identity, start=True, stop=True)
            nc.vector.tensor_copy(yT[b][h * DH:(h + 1) * DH, qo:qo + 512],
                                  ytr[0:DH, 0:4, :])

        def av_norm_half(b, qc, h, py, half, state):
            # last chunk: normalize/transpose/copy one 256-token half so the
            # output projection and stores start before the other half's AV
            rpool, npool = pools["rp"], pools["np"]
            qo = qc * 512
            q0 = 2 * half
            rsb = rpool.tile([128, 2, 1], F32, tag="rs2", name="rs2")
            nc.vector.reciprocal(rsb, py[:, q0:q0 + 2, DH:DV])
            yn = npool.tile([128, 2, DH], BF16, tag="yn2", name="yn2")
            nc.vector.tensor_tensor(yn, py[:, q0:q0 + 2, 0:DH],
                                    rsb.to_broadcast((128, 2, DH)),
                                    mybir.AluOpType.mult)
            if half == 0:
                state["ytr"] = pools["stp"].tile([128, 8, 128], F32,
                                                 tag="st", name="ytr_l")
            ytr = state["ytr"]
            for qt in (q0, q0 + 1):
                nc.tensor.matmul(ytr[0:DH, qt, :], lhsT=yn[:, qt - q0, :],
                                 rhs=identity, start=True, stop=True)
            nc.vector.tensor_copy(
                yT[b][h * DH:(h + 1) * DH, qo + half * 256:qo + half * 256 + 256],
                ytr[0:DH, q0:q0 + 2, :])

        def av_piece_list(b, qc, h, split=False):
            state = {}

            def alloc_g0():
                state["py"] = pools["pyp"].tile(
                    [128, 4, 128], F32, tag=f"py{h}", name=f"py{h}")
                av_group(b, qc, h, 0, state["py"])

            if split:
                return [
                    alloc_g0,
                    lambda: av_group(b, qc, h, 1, state["py"]),
                    lambda: av_norm_half(b, qc, h, state["py"], 0, state),
                    lambda: av_group(b, qc, h, 2, state["py"]),
                    lambda: av_group(b, qc, h, 3, state["py"]),
                    lambda: av_norm_half(b, qc, h, state["py"], 1, state),
                ]
            return [
                alloc_g0,
                lambda: av_group(b, qc, h, 1, state["py"]),
                lambda: av_group(b, qc, h, 2, state["py"]),
                lambda: (av_group(b, qc, h, 3, state["py"]),
                         av_norm(b, qc, h, state["py"])),
            ]

        proj_ps = {}
        proj_pp = {}

        def proj_piece(b, cc, ot, pool_copy=False, wide=False):
            # ot pairs (0,1), (2,3), ... share one staging tile and one DMA
            cs = slice(cc * CW, (cc + 1) * CW)
            if ot % 2 == 0:
                pp = pools["psV"].tile([128, CW], F32, tag="pp", name="pp")
            else:
                pp = pools["psA"].tile([128, CW], F32, tag="pqk", name="pp")
            nc.tensor.matmul(pp, lhsT=wo_sb[:, ot * 128:(ot + 1) * 128],
                             rhs=yT[b][:, cs],
                             start=True, stop=True)
            if ot % 2 == 0:
                ps = pools["pp"].tile([128, 2, CW], BF16, tag="ps", name="ps")
                proj_ps[(b, cc)] = ps
            else:
                ps = proj_ps[(b, cc)]
            if pool_copy:
                nc.scalar.copy(ps[:, ot % 2, :], pp)
            else:
                nc.vector.tensor_copy(ps[:, ot % 2, :], pp)
            if ot % 2 == 1:
                po = ot - 1
                dst = out_p[po * 128:(po + 2) * 128,
                            b * N + cc * CW:b * N + (cc + 1) * CW]
                nc.sync.dma_start(dst.rearrange("(o p) n -> p o n", p=128), ps)

        def proj_piece_half(b, cc, ot, half):
            # last chunk: 256-token halves, stores fire per (pair, half)
            ts = slice(cc * CW + half * 256, cc * CW + half * 256 + 256)
            if ot % 2 == 0:
                pp = pools["psV"].tile([128, CW], F32, tag="pp", name="pp")
            else:
                pp = pools["psA"].tile([128, CW], F32, tag="pqk", name="pp")
            nc.tensor.matmul(pp[:, 0:256],
                             lhsT=wo_sb[:, ot * 128:(ot + 1) * 128],
                             rhs=yT[b][:, ts],
                             start=True, stop=True)
            if ot % 2 == 0 and half == 0:
                ps = pools["pp"].tile([128, 2, CW], BF16, tag="ps", name="ps")
                proj_ps[(b, cc, ot // 2)] = ps
            else:
                ps = proj_ps[(b, cc, ot // 2)]
            if ot % 2 == 1:
                nc.scalar.copy(ps[:, 1, half * 256:half * 256 + 256],
                               pp[:, 0:256])
            else:
                nc.vector.tensor_copy(ps[:, 0, half * 256:half * 256 + 256],
                                      pp[:, 0:256])
            if ot % 2 == 1:
                po = ot - 1
                dst = out_p[po * 128:(po + 2) * 128, b * N + ts.start:
                            b * N + ts.stop]
                nc.sync.dma_start(dst.rearrange("(o p) n -> p o n", p=128),
                                  ps[:, :, half * 256:half * 256 + 256])

        # ---- split stage-A chunk emitters for lane scheduling -----------
        xts = {}

        def a_pk(b, c, first=False):
            xpool, psA = pools["xp"], pools["psA"]
            cg = b * (CH // B) + c
            xt = xpool.tile([128, KTD, CW], BF16, tag="xt", name="xt")
            xts[(b, c)] = xt
            xs = xT[:, cg * CW:(cg + 1) * CW].rearrange(
                "(ko p) n -> p ko n", p=128)
            nc.sync.dma_start(xt[:, 0:KTD // 2, :], xs[:, 0:KTD // 2, :])
            nc.sync.dma_start(xt[:, KTD // 2:, :], xs[:, KTD // 2:, :])
            pk = psA.tile([128, CW], F32, tag="pqk", name="pk")
            for k in range(KTD):
                nc.tensor.matmul(pk, lhsT=wk_sb[:, k, :], rhs=xt[:, k, :],
                                 start=(k == 0), stop=(k == KTD - 1))
            nc.vector.tensor_tensor(KT[b][c], pk,
                                    bk_sb.to_broadcast((128, CW)),
                                    mybir.AluOpType.add)

        def a_pq(b, c):
            psA = pools["psA"]
            xt = xts[(b, c)]
            pq = psA.tile([128, CW], F32, tag="pqk", name="pq")
            for k in range(KTD):
                nc.tensor.matmul(pq, lhsT=wq_sb[:, k, :], rhs=xt[:, k, :],
                                 start=(k == 0), stop=(k == KTD - 1))
            nc.vector.tensor_tensor(QT[b][c], pq,
                                    bq_sb.to_broadcast((128, CW)),
                                    mybir.AluOpType.add)

        def a_pv(b, c, half):
            psV = pools["psV"]
            xt = xts[(b, c)]
            pv = psV.tile([128, 2, 128], F32, tag="pp", name="pv")
            for si, s in enumerate((2 * half, 2 * half + 1)):
                for k in range(KTD):
                    nc.tensor.matmul(pv[:, si, :],
                                     lhsT=xt[:, k, s * 128:(s + 1) * 128],
                                     rhs=wv_sb[:, k, :],
                                     start=(k == 0), stop=(k == KTD - 1))
            for h in range(HC):
                nc.vector.tensor_copy(
                    V1[b][c][:, 2 * half:2 * half + 2, h, 0:DH],
                    pv[:, :, h * DH:(h + 1) * DH])

        # ---- full emission: EDF-woven unit/piece stream -----------------
        with tc.tile_pool(name="xp", bufs=4) as xpool, \
             tc.tile_pool(name="psA", bufs=1, space="PSUM") as psA, \
             tc.tile_pool(name="psV", bufs=1, space="PSUM") as psV, \
             tc.tile_pool(name="stp", bufs=2, space="PSUM") as stp, \
             tc.tile_pool(name="pyp", bufs=1, space="PSUM") as pyp, \
             tc.tile_pool(name="ep", bufs=int(os.environ.get("KV_EP", 46))) as epool, \
             tc.tile_pool(name="rp", bufs=4) as rpool, \
             tc.tile_pool(name="np", bufs=4) as npool, \
             tc.tile_pool(name="pp", bufs=8) as ppool:
            pools.update(xp=xpool, psA=psA, psV=psV, stp=stp, pyp=pyp,
                         ep=epool, rp=rpool, np=npool, pp=ppool)

            # Global unit order: qc0 of each batch kc-major (so stage-A K
            # tiles feed them incrementally); qc1-3 h-major.
            units = []
            unit_idx = {}
            for b in range(B):
                base = 64 * b
                for kc in range(4):
                    hg = ([(0, 0), (1, 0), (0, 1), (1, 1)]
                          if (b, kc) == (0, 0)
                          else [(h, g) for h in range(HC) for g in range(2)])
                    for h, g in hg:
                        unit_idx[(b, 0, h, 2 * kc + g)] = len(units)
                        units.append((b, 0, h, 2 * kc + g))
                for qc in (1, 2, 3):
                    for h in range(HC):
                        for ktg in range(NKT // 2):
                            unit_idx[(b, qc, h, ktg)] = len(units)
                            units.append((b, qc, h, ktg))

            # warmup exp on a dedicated tiny tile: hoists the ACT table
            # load off the first real exp's critical path
            nc.vector.memset(warm_i, 0.0)
            nc.scalar.activation(warm_o, warm_i, AF.Exp)

            # exp engine split: U_ACT units on ACT spread evenly, rest DVE
            n_dve = 128 - U_ACT
            acc = 0.0
            for i in range(128):
                acc += n_dve / 128.0
                if acc >= 1.0:
                    acc -= 1.0
                    exp_on_act[i] = False
                else:
                    exp_on_act[i] = True

            # Lanes: ordered piece lists [(E, L, pe_cost_ns, fn), ...].
            # E = earliest slot, L = latest slot (deadline for EDF).
            lanes = []

            def a_lane(b):
                lane = []
                off = 64 * b
                if b == 0:
                    # c0's pk/pq pre-emitted before unit 0
                    lane.append((0, 14, 850, lambda: a_pv(0, 0, 0)))
                    lane.append((0, 15, 850, lambda: a_pv(0, 0, 1)))
                    for c in (1, 2, 3):
                        lane.append((0, 4 * c - 1, 1700,
                                     lambda c=c: a_pk(0, c)))
                    for c in (1, 2, 3):
                        for hf in (0, 1):
                            lane.append((1, 16, 850,
                                         lambda c=c, hf=hf: a_pv(0, c, hf)))
                    for c in (1, 2, 3):
                        lane.append((2, 16 * c - 2, 1700,
                                     lambda c=c: a_pq(0, c)))
                else:
                    E = int(os.environ.get('KV_BE', 20))
                    lane.append((E, 59, 1700, lambda: a_pk(1, 0)))
                    lane.append((E + 2, 62, 1700, lambda: a_pq(1, 0)))
                    for c in (1, 2, 3):
                        lane.append((E + 4 * c, 63 + 4 * c, 1700,
                                     lambda c=c: a_pk(1, c)))
                    for c in range(4):
                        for hf in (0, 1):
                            lane.append((E + 16 + c, 76, 850,
                                         lambda c=c, hf=hf: a_pv(1, c, hf)))
                    for c in (1, 2, 3):
                        lane.append((E + 20 + c, 63 + 16 * c, 1700,
                                     lambda c=c: a_pq(1, c)))
                return lane

            def av_lane(h):
                lane = []
                for b in range(B):
                    for qc in range(QC):
                        last = max(unit_idx[(b, qc, h, ktg)]
                                   for ktg in range(NKT // 2))
                        E = last + EXP_SLACK
                        lc = False
                        ps = av_piece_list(b, qc, h, split=lc)
                        if lc:
                            costs = (430, 430, 450, 430, 430, 450)
                            for i, p in enumerate(ps):
                                lane.append((E + i // 2, E + 5 + i, costs[i],
                                             p))
                        else:
                            lane.append((E, E + 5, 120, ps[0]))
                            lane.append((E, E + 6, 120, ps[1]))
                            lane.append((E + 1, E + 7, 120, ps[2]))
                            lane.append((E + 1, E + 8, 430, ps[3]))
                return lane

            def proj_lane():
                lane = []
                for b in range(B):
                    for qc in range(QC):
                        last = max(unit_idx[(b, qc, h, ktg)]
                                   for h in range(HC)
                                   for ktg in range(NKT // 2))
                        E = last + EXP_SLACK + int(os.environ.get('KV_PE', 12))
                        lc0 = (b == B - 1 and qc == QC - 1)
                        lc = False
                        if lc:
                            for half in range(2):
                                for ot in range(D // 128):
                                    lane.append(
                                        (E + half, E + 10 + half * 8 + ot,
                                         110,
                                         lambda ot=ot, half=half:
                                             proj_piece_half(1, 3, ot, half)))
                        else:
                            for ot in range(D // 128):
                                lane.append(
                                    (E + ot // 2, E + 10 + ot, 210,
                                     lambda b=b, qc=qc, ot=ot, lc0=lc0:
                                         proj_piece(b, qc, ot,
                                                    pool_copy=(lc0 and
                                                               ot % 2 == 1))))
                return lane

            lanes.append(a_lane(0))
            lanes.append(a_lane(1))
            lanes.append(av_lane(0))
            lanes.append(av_lane(1))
            lanes.append(proj_lane())
            heads = [0] * len(lanes)

            # PE warmup: dummy zero matmuls keep the tensor engine busy
            # through the p-state ramp while the first x chunk loads, so the
            # first real projections run at full clock.
            zeros = singles.tile([128, CW], BF16, name="zeros")
            nc.vector.memset(zeros, 0.0)
            for w in range(0):
                wps = pools["psA"].tile([128, CW], F32, tag="pqk", name="wps")
                nc.tensor.matmul(wps, lhsT=zeros[:, 0:128], rhs=zeros,
                                 start=True, stop=True)

            # head: c0 q/k before unit 0. DMA order is the critical path to
            # the first exp: stream the first two k-tiles of wq/wk, then the
            # x quarters, then the weight remainders; wv/wo/biases follow.
            xpool0 = pools["xp"]
            xt0 = xpool0.tile([128, KTD, CW], BF16, tag="xt", name="xt")
            xts[(0, 0)] = xt0
            xs0 = xT[:, 0:CW].rearrange("(ko p) n -> p ko n", p=128)
            wqr = wq.rearrange("(ko p) m -> p ko m", p=128)
            wkr = wk.rearrange("(ko p) m -> p ko m", p=128)
            nc.sync.dma_start(wq_sb[:, 0:2, :], wqr[:, 0:2, :])
            nc.sync.dma_start(wk_sb[:, 0:2, :], wkr[:, 0:2, :])
            nc.sync.dma_start(xt0[:, 0:2, :], xs0[:, 0:2, :])
            nc.sync.dma_start(xt0[:, 2:4, :], xs0[:, 2:4, :])
            nc.sync.dma_start(wq_sb[:, 2:KTD, :], wqr[:, 2:KTD, :])
            nc.sync.dma_start(wk_sb[:, 2:KTD, :], wkr[:, 2:KTD, :])
            for qq in range(2, 4):
                nc.sync.dma_start(xt0[:, 2 * qq:2 * qq + 2, :],
                                  xs0[:, 2 * qq:2 * qq + 2, :])
            nc.sync.dma_start(bq_sb, bq)
            nc.sync.dma_start(bk_sb, bk)
            nc.sync.dma_start(wv_sb, wv.rearrange("(ko p) m -> p ko m", p=128))
            nc.sync.dma_start(wo_sb, wo)
            # interleave the k-accumulations of pq0/pk0 across two banks so
            # QT/KT c0 land together ~3us sooner (first exp gates on both)
            pq0 = pools["psA"].tile([128, CW], F32, tag="pqk", name="pq")
            pk0 = pools["stp"].tile([128, 2, 512], F32, tag="st",
                                    name="pk")[:, 0, :]
            for k in range(KTD):
                nc.tensor.matmul(pq0, lhsT=wq_sb[:, k, :], rhs=xt0[:, k, :],
                                 start=(k == 0), stop=(k == KTD - 1))
                nc.tensor.matmul(pk0, lhsT=wk_sb[:, k, :], rhs=xt0[:, k, :],
                                 start=(k == 0), stop=(k == KTD - 1))
            nc.vector.tensor_tensor(QT[0][0], pq0,
                                    bq_sb.to_broadcast((128, CW)),
                                    mybir.AluOpType.add)
            # KT bias on ACT (idle before the first exp) so the two c0 bias
            # adds run in parallel instead of serializing on DVE
            nc.scalar.activation(KT[0][0], pk0, AF.Identity, bias=bk_sb)

            # EDF weave
            UNIT_COST = 430.0
            TARGET = float(os.environ.get("KV_TARGET", 780.0))
            debt = 0.0
            for idx, (b, qc, h, ktg) in enumerate(units):
                sc_unit(idx, b, qc, h, ktg)
                budget = TARGET - UNIT_COST + debt
                while True:
                    best = None
                    urgent = False
                    for li, lane in enumerate(lanes):
                        if heads[li] >= len(lane):
                            continue
                        E, L, cost, fn = lane[heads[li]]
                        if E > idx:
                            continue
                        if L <= idx + 1:
                            best, urgent = li, True
                            break
                        if best is None or L < lanes[best][heads[best]][1]:
                            best = li
                    if best is None:
                        break
                    E, L, cost, fn = lanes[best][heads[best]]
                    if not urgent and budget < cost * float(os.environ.get('KV_BF', 0.5)):
                        break
                    fn()
                    heads[best] += 1
                    budget -= cost
                debt = min(max(budget, -2000.0), 500.0)
            # drain remaining pieces in deadline order
            rest = []
            for li, lane in enumerate(lanes):
                rest.extend(lane[heads[li]:])
            rest.sort(key=lambda p: p[1])
            for E, L, cost, fn in rest:
                fn()

    nc.compile()
    return nc


_CACHE = {}


def _get_nc():
    if "nc" not in _CACHE:
        _CACHE["nc"] = build_nc()
    return _CACHE["nc"]


def _prep_inputs(x, Wqkv, bqkv):
    bf = ml_dtypes.bfloat16
    x = np.asarray(x, np.float32)
    Wqkv = np.asarray(Wqkv, np.float32)
    bqkv = np.asarray(bqkv, np.float32)
    xT = np.ascontiguousarray(x.reshape(TOK, D).T).astype(bf)
    in_maps = []
    for c in range(NC):
        cs = slice(c * 128, (c + 1) * 128)
        in_maps.append({
            "xT": xT,
            "wq": np.ascontiguousarray(Wqkv[:, 0 * D + c * 128:0 * D + (c + 1) * 128]).astype(bf),
            "wk": np.ascontiguousarray(Wqkv[:, 1 * D + c * 128:1 * D + (c + 1) * 128]).astype(bf),
            "wv": np.ascontiguousarray(Wqkv[:, 2 * D + c * 128:2 * D + (c + 1) * 128]).astype(bf),
            "wo": None,  # filled by caller (needs Wproj)
            "bq": np.ascontiguousarray(bqkv[0 * D + c * 128:0 * D + (c + 1) * 128]).reshape(128, 1).astype(np.float32),
            "bk": np.ascontiguousarray(bqkv[1 * D + c * 128:1 * D + (c + 1) * 128]).reshape(128, 1).astype(np.float32),
        })
    return in_maps


def _run(x, Wqkv, bqkv, Wproj, bproj, trace=False):
    bf = ml_dtypes.bfloat16
    Wproj = np.asarray(Wproj, np.float32)
    bproj = np.asarray(bproj, np.float32)
    bqkv_np = np.asarray(bqkv, np.float32)
    in_maps = _prep_inputs(x, Wqkv, bqkv_np)
    for c in range(NC):
        in_maps[c]["wo"] = np.ascontiguousarray(
            Wproj[c * 128:(c + 1) * 128, :]).astype(bf)
    nc = _get_nc()
    res = run_bass_kernel_spmd(nc, in_maps, core_ids=list(range(NC)), trace=trace)
    acc = res.results[0]["out_p"].astype(np.float32).copy()
    for c in range(1, NC):
        acc += res.results[c]["out_p"]
    bv = bqkv_np[2 * D:]
    bias_eff = (bv @ Wproj + bproj).astype(np.float32)
    out = np.ascontiguousarray(acc.T).reshape(B, N, D) + bias_eff
    return out.astype(np.float32), res


def kernel(x, Wqkv, bqkv, Wproj, bproj):
    out, _ = _run(x, Wqkv, bqkv, Wproj, bproj, trace=False)
    return out

